# revision 3
# baseline (speedup 1.0000x reference)
"""Trainium2 Bass kernel: binarized-MLP forward (784-256-128-32-10, ste_sign).

Strategy
--------
Pure data parallel over 8 NeuronCores: batch 65536 -> 8 shards of 8192 rows;
the tiny sign-binarized weights are replicated (binarized + transposed on the
host). Each core runs the full 4-layer network on its shard; outputs are
gathered on the host. No collectives needed (forward only).

On-chip the network runs feature-major: activations live as [features, batch]
tiles and every matmul streams batch as the moving dimension, so layer N's
output feeds layer N+1 with no transposes between layers. x is pre-transposed
to [784, B] on the host so the contraction dim lands on the partition axis
straight out of DMA.

Layer 1 (x is real-valued fp32; everything downstream only sees sign(h1)) uses
a two-pass split exploiting the PE's float32r mode: hardware f32r rounds each
input RNE to 11 explicit mantissa bits and accumulates in fp32 at 1 cycle/row
(vs 4 for fp32). The host ships xq = rne11(x) (consumed as f32r -> exact) plus
the bf16-rounded residual (x - xq); both passes accumulate into one PSUM
group, recovering ~21 mantissa bits — beyond the fp32 reference's own rounding
noise — at half the PE cost of native fp32 (measured on HW: max err 3.3e-5 on
784-length dots, 0 sign flips in 65536).

Layers 2-4 have +-1 inputs and +-1 weights, so bf16 matmuls are exact
(integer partial sums <= 256); the sign activations run on the scalar engine.
ACT Sign(0) = 0 on this HW, so integer-valued pre-activations (layers 2,3) use
Sign(h + 0.5), which reproduces the reference's sign(0)=+1 exactly. The final
logits are integers in [-32, 32], computed exactly.

x is loaded in [k-tile, 1024]-column super-tiles (fewer, larger DMAs — the
DMA queue is the second-busiest resource), split across both HWDGE engines
(xq on SP, residual on ACT), with the first super-group prefetched ahead of
the weight loads so the PE starts early.

This walrus build rejects instructions carrying more than one semaphore wait
("Too many sync wait commands"), so after Tile scheduling, excess waits are
split onto preceding same-engine NoOps (fix_sync_waits).
"""
import sys
sys.path.insert(0, '/opt/trn_rl_repo')
import numpy as np
import ml_dtypes
import concourse.bass as bass
import concourse.mybir as mybir
from concourse import tile
from concourse.bass_utils import run_bass_kernel_spmd

BF16 = ml_dtypes.bfloat16
F32 = mybir.dt.float32
F32R = mybir.dt.float32r
BF = mybir.dt.bfloat16
AF = mybir.ActivationFunctionType

N_CORES = 8
B_LOC = 8192          # batch rows per core
NB = 512              # batch columns per compute chunk (one fp32 PSUM bank)
NCHUNK = B_LOC // NB
NB_LOAD = 1024        # batch columns per x DMA super-tile
K1 = 784
KTILES = [(k, min(128, K1 - k)) for k in range(0, K1, 128)]  # 6x128 + 16
F1, F2, F3, F4 = 256, 128, 32, 10
MAX_WAITS = 1
PASS_DT = ((F32R, np.float32), (BF, BF16))   # L1 passes: f32r, then bf16 residual


def fix_sync_waits(nc):
    for fn in nc.m.functions:
        for bb in fn.blocks:
            out = []
            changed = False
            for ins in bb.instructions:
                si = ins.sync_info
                waits = list(si.on_wait) if si is not None else []
                if len(waits) > MAX_WAITS:
                    head, keep = waits[:-MAX_WAITS], waits[-MAX_WAITS:]
                    k = 0
                    while head:
                        chunk, head = head[:MAX_WAITS], head[MAX_WAITS:]
                        nop = mybir.InstNoOp(
                            name=f"{ins.name}-wsplit{k}", engine=ins.engine)
                        nop.sync_info = mybir.SyncInfo(on_wait=chunk, on_update=[])
                        out.append(nop)
                        k += 1
                    ins.sync_info = mybir.SyncInfo(
                        on_wait=keep, on_update=list(si.on_update))
                    changed = True
                out.append(ins)
            if changed:
                bb.instructions = out


def round_mant11(a):
    """fp32 -> RNE at 11 explicit mantissa bits (= HW f32r input rounding)."""
    u = np.ascontiguousarray(a).view(np.uint32)
    drop = 12
    lsb = ((u >> drop) & 1).astype(np.uint32)
    r = ((u + np.uint32((1 << (drop - 1)) - 1) + lsb) >> drop) << drop
    return r.view(np.float32)


def build_nc(rep=1):
    nc = bass.Bass()
    x_in = [nc.declare_dram_parameter(nm, [K1, B_LOC], dt, isOutput=False)
            for nm, (dt, _) in zip(("xq", "xs"), PASS_DT)]
    w1_d = [nc.declare_dram_parameter(f"w1sT{pi}", [K1, F1], dt, isOutput=False)
            for pi, (dt, _) in enumerate(PASS_DT)]
    w2_d = nc.declare_dram_parameter("w2sT", [F1, F2], BF, isOutput=False)
    w3_d = nc.declare_dram_parameter("w3sT", [F2, F3], BF, isOutput=False)
    w4_d = nc.declare_dram_parameter("w4sT", [F3, F4], BF, isOutput=False)
    out_d = nc.declare_dram_parameter("out", [F4, B_LOC], F32, isOutput=True)

    with tile.TileContext(nc) as tc:
        with tc.tile_pool(name="wpool", bufs=1) as wpool, \
             tc.tile_pool(name="xtpool", bufs=2) as xtpool, \
             tc.tile_pool(name="apool", bufs=2) as apool, \
             tc.tile_pool(name="opool", bufs=2) as opool, \
             tc.tile_pool(name="ps1", bufs=2, space="PSUM") as ps1, \
             tc.tile_pool(name="ps2", bufs=2, space="PSUM") as ps2, \
             tc.tile_pool(name="ps34", bufs=1, space="PSUM") as ps34:
            # prefetch first x super-group ahead of the weights (fast PE start)
            xg0 = [None, None]
            for pi, (dt, _) in enumerate(PASS_DT):
                row = []
                for i, (k0, kw) in enumerate(KTILES):
                    t = xtpool.tile([kw, NB_LOAD], dt,
                                    name=f"xG_0_0_{pi}_{i}", tag=f"xG{pi}{i}")
                    eng = nc.scalar if pi == 1 else nc.sync
                    eng.dma_start(t[:], x_in[pi][k0:k0 + kw, 0:NB_LOAD])
                    row.append(t)
                xg0[pi] = row
            # resident weights / constants
            w1_t = []
            for pi, (dt, _) in enumerate(PASS_DT):
                row = []
                for i, (k0, kw) in enumerate(KTILES):
                    t = wpool.tile([kw, F1], dt, name=f"w1t{pi}_{i}")
                    nc.sync.dma_start(t[:], w1_d[pi][k0:k0 + kw, :])
                    row.append(t)
                w1_t.append(row)
            w2_t = []
            for i in range(2):
                t = wpool.tile([128, F2], BF, name=f"w2t{i}")
                nc.sync.dma_start(t[:], w2_d[i * 128:(i + 1) * 128, :])
                w2_t.append(t)
            w3_t = wpool.tile([F2, F3], BF, name="w3t")
            nc.sync.dma_start(w3_t[:], w3_d[:, :])
            w4_t = wpool.tile([F3, F4], BF, name="w4t")
            nc.sync.dma_start(w4_t[:], w4_d[:, :])
            zb = wpool.tile([128, 1], F32, name="zb")
            nc.vector.memset(zb[:], 0.0)
            hb = wpool.tile([128, 1], F32, name="hb")
            nc.vector.memset(hb[:], 0.5)

            nsub = NB_LOAD // NB
            xg = [None, None]
            for r in range(rep):
                for c in range(NCHUNK):
                    b0 = c * NB
                    g, j = divmod(c, nsub)
                    gb0 = g * NB_LOAD
                    if r == 0 and g == 0:
                        if j == 0:
                            xg = list(xg0)
                    elif j == 0:
                        for pi, (dt, _) in enumerate(PASS_DT):
                            row = []
                            for i, (k0, kw) in enumerate(KTILES):
                                t = xtpool.tile([kw, NB_LOAD], dt,
                                                name=f"xG_{r}_{g}_{pi}_{i}",
                                                tag=f"xG{pi}{i}")
                                eng = nc.scalar if pi == 1 else nc.sync
                                eng.dma_start(
                                    t[:], x_in[pi][k0:k0 + kw, gb0:gb0 + NB_LOAD])
                                row.append(t)
                            xg[pi] = row
                    xT = [[t[:, j * NB:(j + 1) * NB] for t in xg[pi]]
                          for pi in range(2)]
                    # ---- L1: 2 f-halves, accumulate f32r pass then residual pass
                    a1 = []
                    for f in range(2):
                        p1 = ps1.tile([128, NB], F32, name=f"p1_{r}_{c}_{f}",
                                      tag="p1")
                        nmm = 2 * len(KTILES)
                        mj = 0
                        for pi in range(2):
                            for i in range(len(KTILES)):
                                nc.tensor.matmul(
                                    p1[:], w1_t[pi][i][:, f * 128:(f + 1) * 128],
                                    xT[pi][i],
                                    start=(mj == 0), stop=(mj == nmm - 1))
                                mj += 1
                        s1 = apool.tile([128, NB], BF, name=f"a1_{r}_{c}_{f}",
                                        tag=f"a1{f}")
                        nc.scalar.activation(s1[:], p1[:], AF.Sign, bias=zb[:],
                                             scale=1.0)
                        a1.append(s1)
                    # ---- L2 (bf16 exact)
                    p2 = ps2.tile([F2, NB], F32, name=f"p2_{r}_{c}", tag="p2")
                    nc.tensor.matmul(p2[:], w2_t[0][:], a1[0][:], start=True,
                                     stop=False)
                    nc.tensor.matmul(p2[:], w2_t[1][:], a1[1][:], start=False,
                                     stop=True)
                    a2 = apool.tile([F2, NB], BF, name=f"a2_{r}_{c}", tag="a2")
                    nc.scalar.activation(a2[:], p2[:], AF.Sign, bias=hb[:],
                                         scale=1.0)
                    # ---- L3
                    p3 = ps34.tile([F3, NB], F32, name=f"p3_{r}_{c}", tag="p3")
                    nc.tensor.matmul(p3[:], w3_t[:], a2[:], start=True, stop=True)
                    a3 = apool.tile([F3, NB], BF, name=f"a3_{r}_{c}", tag="a3")
                    nc.scalar.activation(a3[:], p3[:], AF.Sign, bias=hb[:F3, :],
                                         scale=1.0)
                    # ---- L4 (integer logits, exact)
                    p4 = ps34.tile([F4, NB], F32, name=f"p4_{r}_{c}", tag="p4")
                    nc.tensor.matmul(p4[:], w4_t[:], a3[:], start=True, stop=True)
                    o = opool.tile([F4, NB], F32, name=f"o_{r}_{c}", tag="o")
                    nc.vector.tensor_copy(o[:], p4[:])
                    nc.sync.dma_start(out_d[:, b0:b0 + NB], o[:])
    fix_sync_waits(nc)
    return nc


def _sg(w):
    return np.where(w >= 0, np.float32(1.0), np.float32(-1.0))


_NC_CACHE = {}


def kernel(x, w1, w2, w3, w4):
    if "nc" not in _NC_CACHE:
        _NC_CACHE["nc"] = build_nc()
    nc = _NC_CACHE["nc"]

    x = np.ascontiguousarray(np.asarray(x).reshape(-1, K1), dtype=np.float32)
    w1sT = np.ascontiguousarray(_sg(np.asarray(w1)).T)
    wm = {
        "w1sT0": w1sT.astype(np.float32),      # consumed as f32r (+-1 exact)
        "w1sT1": w1sT.astype(BF16),
        "w2sT": np.ascontiguousarray(_sg(np.asarray(w2)).T).astype(BF16),
        "w3sT": np.ascontiguousarray(_sg(np.asarray(w3)).T).astype(BF16),
        "w4sT": np.ascontiguousarray(_sg(np.asarray(w4)).T).astype(BF16),
    }
    xq = round_mant11(x)
    xs = (x - xq).astype(BF16)
    xqT = np.ascontiguousarray(xq.T)           # [784, 65536]
    xsT = np.ascontiguousarray(xs.T)

    maps = []
    for c in range(N_CORES):
        m = dict(wm)
        m["xq"] = xqT[:, c * B_LOC:(c + 1) * B_LOC]
        m["xs"] = xsT[:, c * B_LOC:(c + 1) * B_LOC]
        maps.append(m)

    res = None
    last_exc = None
    for attempt in range(3):
        try:
            res = run_bass_kernel_spmd(nc, maps, list(range(N_CORES)))
            break
        except Exception as e:  # transient NRT/device errors: retry
            last_exc = e
            import time
            time.sleep(5 * (attempt + 1))
    if res is None:
        raise last_exc
    outs = [r["out"] for r in res.results]                 # [10, 8192] each
    return np.ascontiguousarray(
        np.concatenate([o.T for o in outs], axis=0)).astype(np.float32)


# revision 4
# speedup vs baseline: 1.0228x; 1.0228x over previous
"""Trainium2 Bass kernel: binarized-MLP forward (784-256-128-32-10, ste_sign).

Strategy
--------
Pure data parallel over 8 NeuronCores: batch 65536 -> 8 shards of 8192 rows;
the tiny sign-binarized weights are replicated (binarized + transposed on the
host). Each core runs the full 4-layer network on its shard; outputs are
gathered on the host. No collectives needed (forward only).

On-chip the network runs feature-major: activations live as [features, batch]
tiles and every matmul streams batch as the moving dimension, so layer N's
output feeds layer N+1 with no transposes between layers. x is pre-transposed
to [784, B] on the host so the contraction dim lands on the partition axis
straight out of DMA.

Layer 1 (x is real-valued fp32; everything downstream only sees sign(h1)) uses
a Dekker-style two-pass fp16 split: the host ships xh = fp16(x) plus the
fp16-rounded residual (x - xh), and both passes (weights +-1, exact in fp16)
accumulate into one PSUM group at 1 cycle/row each (vs 4 for native fp32).
This recovers ~21 mantissa bits — beyond the fp32 reference's own rounding
noise (measured on HW: max err 2.1e-5 on 784-length dots, 0 sign flips in
65536) — at half the PE cost of fp32 and 4 bytes/element of DMA (fp32's
bandwidth) instead of 6 for an f32r+bf16 split.

Layers 2-4 have +-1 inputs and +-1 weights, so bf16 matmuls are exact
(integer partial sums <= 256); the sign activations run on the scalar engine.
ACT Sign(0) = 0 on this HW, so integer-valued pre-activations (layers 2,3) use
Sign(h + 0.5), which reproduces the reference's sign(0)=+1 exactly. The final
logits are integers in [-32, 32], computed exactly.

x is loaded in [k-tile, 1024]-column super-tiles (fewer, larger DMAs — the
DMA queue is the second-busiest resource), split across both HWDGE engines
(xq on SP, residual on ACT), with the first super-group prefetched ahead of
the weight loads so the PE starts early.

This walrus build rejects instructions carrying more than one semaphore wait
("Too many sync wait commands"), so after Tile scheduling, excess waits are
split onto preceding same-engine NoOps (fix_sync_waits).
"""
import sys
sys.path.insert(0, '/opt/trn_rl_repo')
import numpy as np
import ml_dtypes
import concourse.bass as bass
import concourse.mybir as mybir
from concourse import tile
from concourse.bass_utils import run_bass_kernel_spmd

BF16 = ml_dtypes.bfloat16
F32 = mybir.dt.float32
FP16 = mybir.dt.float16
BF = mybir.dt.bfloat16
AF = mybir.ActivationFunctionType

N_CORES = 8
B_LOC = 8192          # batch rows per core
NB = 512              # batch columns per compute chunk (one fp32 PSUM bank)
NCHUNK = B_LOC // NB
NB_LOAD = 1024        # batch columns per x DMA super-tile
K1 = 784
KTILES = [(k, min(128, K1 - k)) for k in range(0, K1, 128)]  # 6x128 + 16
F1, F2, F3, F4 = 256, 128, 32, 10
MAX_WAITS = 1
PASS_DT = ((FP16, np.float16), (FP16, np.float16))  # L1: fp16 hi, fp16 residual


def fix_sync_waits(nc):
    for fn in nc.m.functions:
        for bb in fn.blocks:
            out = []
            changed = False
            for ins in bb.instructions:
                si = ins.sync_info
                waits = list(si.on_wait) if si is not None else []
                if len(waits) > MAX_WAITS:
                    head, keep = waits[:-MAX_WAITS], waits[-MAX_WAITS:]
                    k = 0
                    while head:
                        chunk, head = head[:MAX_WAITS], head[MAX_WAITS:]
                        nop = mybir.InstNoOp(
                            name=f"{ins.name}-wsplit{k}", engine=ins.engine)
                        nop.sync_info = mybir.SyncInfo(on_wait=chunk, on_update=[])
                        out.append(nop)
                        k += 1
                    ins.sync_info = mybir.SyncInfo(
                        on_wait=keep, on_update=list(si.on_update))
                    changed = True
                out.append(ins)
            if changed:
                bb.instructions = out


def round_mant11(a):
    """fp32 -> RNE at 11 explicit mantissa bits (= HW f32r input rounding)."""
    u = np.ascontiguousarray(a).view(np.uint32)
    drop = 12
    lsb = ((u >> drop) & 1).astype(np.uint32)
    r = ((u + np.uint32((1 << (drop - 1)) - 1) + lsb) >> drop) << drop
    return r.view(np.float32)


def build_nc(rep=1):
    nc = bass.Bass()
    x_in = [nc.declare_dram_parameter(nm, [K1, B_LOC], dt, isOutput=False)
            for nm, (dt, _) in zip(("xq", "xs"), PASS_DT)]
    w1_d = [nc.declare_dram_parameter(f"w1sT{pi}", [K1, F1], dt, isOutput=False)
            for pi, (dt, _) in enumerate(PASS_DT)]
    w2_d = nc.declare_dram_parameter("w2sT", [F1, F2], BF, isOutput=False)
    w3_d = nc.declare_dram_parameter("w3sT", [F2, F3], BF, isOutput=False)
    w4_d = nc.declare_dram_parameter("w4sT", [F3, F4], BF, isOutput=False)
    out_d = nc.declare_dram_parameter("out", [F4, B_LOC], F32, isOutput=True)

    with tile.TileContext(nc) as tc:
        with tc.tile_pool(name="wpool", bufs=1) as wpool, \
             tc.tile_pool(name="xtpool", bufs=2) as xtpool, \
             tc.tile_pool(name="apool", bufs=2) as apool, \
             tc.tile_pool(name="opool", bufs=2) as opool, \
             tc.tile_pool(name="ps1", bufs=2, space="PSUM") as ps1, \
             tc.tile_pool(name="ps2", bufs=2, space="PSUM") as ps2, \
             tc.tile_pool(name="ps34", bufs=1, space="PSUM") as ps34:
            # head: interleave pass-0 weight k-tiles with the first x
            # super-group so the first matmul's operands land back-to-back.
            xg0 = [[], []]
            w1_t = [[None] * len(KTILES) for _ in PASS_DT]
            for i, (k0, kw) in enumerate(KTILES):
                t = wpool.tile([kw, F1], PASS_DT[0][0], name=f"w1t0_{i}")
                nc.sync.dma_start(t[:], w1_d[0][k0:k0 + kw, :])
                w1_t[0][i] = t
                for pi, (dt, _) in enumerate(PASS_DT):
                    tx = xtpool.tile([kw, NB_LOAD], dt,
                                     name=f"xG_0_0_{pi}_{i}", tag=f"xG{pi}{i}")
                    eng = nc.scalar if pi == 1 else nc.sync
                    eng.dma_start(tx[:], x_in[pi][k0:k0 + kw, 0:NB_LOAD])
                    xg0[pi].append(tx)
            for i, (k0, kw) in enumerate(KTILES):
                t = wpool.tile([kw, F1], PASS_DT[1][0], name=f"w1t1_{i}")
                nc.scalar.dma_start(t[:], w1_d[1][k0:k0 + kw, :])
                w1_t[1][i] = t
            w2_t = []
            for i in range(2):
                t = wpool.tile([128, F2], BF, name=f"w2t{i}")
                nc.sync.dma_start(t[:], w2_d[i * 128:(i + 1) * 128, :])
                w2_t.append(t)
            w3_t = wpool.tile([F2, F3], BF, name="w3t")
            nc.sync.dma_start(w3_t[:], w3_d[:, :])
            w4_t = wpool.tile([F3, F4], BF, name="w4t")
            nc.sync.dma_start(w4_t[:], w4_d[:, :])
            zb = wpool.tile([128, 1], F32, name="zb")
            nc.vector.memset(zb[:], 0.0)
            hb = wpool.tile([128, 1], F32, name="hb")
            nc.vector.memset(hb[:], 0.5)

            nsub = NB_LOAD // NB
            xg = [None, None]
            for r in range(rep):
                for c in range(NCHUNK):
                    b0 = c * NB
                    g, j = divmod(c, nsub)
                    gb0 = g * NB_LOAD
                    if r == 0 and g == 0:
                        if j == 0:
                            xg = list(xg0)
                    elif j == 0:
                        for pi, (dt, _) in enumerate(PASS_DT):
                            row = []
                            for i, (k0, kw) in enumerate(KTILES):
                                t = xtpool.tile([kw, NB_LOAD], dt,
                                                name=f"xG_{r}_{g}_{pi}_{i}",
                                                tag=f"xG{pi}{i}")
                                eng = nc.scalar if pi == 1 else nc.sync
                                eng.dma_start(
                                    t[:], x_in[pi][k0:k0 + kw, gb0:gb0 + NB_LOAD])
                                row.append(t)
                            xg[pi] = row
                    xT = [[t[:, j * NB:(j + 1) * NB] for t in xg[pi]]
                          for pi in range(2)]
                    # ---- L1: 2 f-halves, accumulate f32r pass then residual pass
                    a1 = []
                    for f in range(2):
                        p1 = ps1.tile([128, NB], F32, name=f"p1_{r}_{c}_{f}",
                                      tag="p1")
                        nmm = 2 * len(KTILES)
                        mj = 0
                        for pi in range(2):
                            for i in range(len(KTILES)):
                                nc.tensor.matmul(
                                    p1[:], w1_t[pi][i][:, f * 128:(f + 1) * 128],
                                    xT[pi][i],
                                    start=(mj == 0), stop=(mj == nmm - 1))
                                mj += 1
                        s1 = apool.tile([128, NB], BF, name=f"a1_{r}_{c}_{f}",
                                        tag=f"a1{f}")
                        nc.scalar.activation(s1[:], p1[:], AF.Sign, bias=zb[:],
                                             scale=1.0)
                        a1.append(s1)
                    # ---- L2 (bf16 exact)
                    p2 = ps2.tile([F2, NB], F32, name=f"p2_{r}_{c}", tag="p2")
                    nc.tensor.matmul(p2[:], w2_t[0][:], a1[0][:], start=True,
                                     stop=False)
                    nc.tensor.matmul(p2[:], w2_t[1][:], a1[1][:], start=False,
                                     stop=True)
                    a2 = apool.tile([F2, NB], BF, name=f"a2_{r}_{c}", tag="a2")
                    nc.scalar.activation(a2[:], p2[:], AF.Sign, bias=hb[:],
                                         scale=1.0)
                    # ---- L3
                    p3 = ps34.tile([F3, NB], F32, name=f"p3_{r}_{c}", tag="p3")
                    nc.tensor.matmul(p3[:], w3_t[:], a2[:], start=True, stop=True)
                    a3 = apool.tile([F3, NB], BF, name=f"a3_{r}_{c}", tag="a3")
                    nc.scalar.activation(a3[:], p3[:], AF.Sign, bias=hb[:F3, :],
                                         scale=1.0)
                    # ---- L4 (integer logits, exact)
                    p4 = ps34.tile([F4, NB], F32, name=f"p4_{r}_{c}", tag="p4")
                    nc.tensor.matmul(p4[:], w4_t[:], a3[:], start=True, stop=True)
                    o = opool.tile([F4, NB], F32, name=f"o_{r}_{c}", tag="o")
                    nc.vector.tensor_copy(o[:], p4[:])
                    nc.sync.dma_start(out_d[:, b0:b0 + NB], o[:])
    fix_sync_waits(nc)
    return nc


def _sg(w):
    return np.where(w >= 0, np.float32(1.0), np.float32(-1.0))


_NC_CACHE = {}


def kernel(x, w1, w2, w3, w4):
    if "nc" not in _NC_CACHE:
        _NC_CACHE["nc"] = build_nc()
    nc = _NC_CACHE["nc"]

    x = np.ascontiguousarray(np.asarray(x).reshape(-1, K1), dtype=np.float32)
    w1sT = np.ascontiguousarray(_sg(np.asarray(w1)).T)
    wm = {
        "w1sT0": w1sT.astype(np.float16),      # +-1 exact in fp16
        "w1sT1": w1sT.astype(np.float16),
        "w2sT": np.ascontiguousarray(_sg(np.asarray(w2)).T).astype(BF16),
        "w3sT": np.ascontiguousarray(_sg(np.asarray(w3)).T).astype(BF16),
        "w4sT": np.ascontiguousarray(_sg(np.asarray(w4)).T).astype(BF16),
    }
    xq = x.astype(np.float16)
    xs = (x - xq.astype(np.float32)).astype(np.float16)
    xqT = np.ascontiguousarray(xq.T)           # [784, 65536]
    xsT = np.ascontiguousarray(xs.T)

    maps = []
    for c in range(N_CORES):
        m = dict(wm)
        m["xq"] = xqT[:, c * B_LOC:(c + 1) * B_LOC]
        m["xs"] = xsT[:, c * B_LOC:(c + 1) * B_LOC]
        maps.append(m)

    res = None
    last_exc = None
    for attempt in range(3):
        try:
            res = run_bass_kernel_spmd(nc, maps, list(range(N_CORES)))
            break
        except Exception as e:  # transient NRT/device errors: retry
            last_exc = e
            import time
            time.sleep(5 * (attempt + 1))
    if res is None:
        raise last_exc
    outs = [r["out"] for r in res.results]                 # [10, 8192] each
    return np.ascontiguousarray(
        np.concatenate([o.T for o in outs], axis=0)).astype(np.float32)


# revision 5
# speedup vs baseline: 1.0813x; 1.0572x over previous
"""Trainium2 Bass kernel: binarized-MLP forward (784-256-128-32-10, ste_sign).

Strategy
--------
Pure data parallel over 8 NeuronCores: batch 65536 -> 8 shards of 8192 rows;
the tiny sign-binarized weights are replicated (binarized + transposed on the
host). Each core runs the full 4-layer network on its shard; outputs are
gathered on the host. No collectives needed (forward only).

On-chip the network runs feature-major: activations live as [features, batch]
tiles and every matmul streams batch as the moving dimension, so layer N's
output feeds layer N+1 with no transposes between layers. x is pre-transposed
to [784, B] on the host so the contraction dim lands on the partition axis
straight out of DMA.

Layer 1 (x is real-valued fp32; everything downstream only sees sign(h1)) uses
a Dekker-style two-pass fp16 split: the host ships xh = fp16(x) plus the
fp16-rounded residual (x - xh), and both passes (weights +-1, exact in fp16)
accumulate into one PSUM group at 1 cycle/row each (vs 4 for native fp32).
This recovers ~21 mantissa bits — beyond the fp32 reference's own rounding
noise (measured on HW: max err 2.1e-5 on 784-length dots, 0 sign flips in
65536) — at half the PE cost of fp32 and 4 bytes/element of DMA (fp32's
bandwidth) instead of 6 for an f32r+bf16 split.

Layers 2-4 have +-1 inputs and +-1 weights, so bf16 matmuls are exact
(integer partial sums <= 256); the sign activations run on the scalar engine.
ACT Sign(0) = 0 on this HW, so integer-valued pre-activations (layers 2,3) use
Sign(h + 0.5), which reproduces the reference's sign(0)=+1 exactly. The final
logits are integers in [-32, 32], computed exactly.

x is loaded in [k-tile, 1024]-column super-tiles (fewer, larger DMAs — the
DMA queue is the second-busiest resource), split across both HWDGE engines
(xq on SP, residual on ACT), with the first super-group prefetched ahead of
the weight loads so the PE starts early.

This walrus build rejects instructions carrying more than one semaphore wait
("Too many sync wait commands"), so after Tile scheduling, excess waits are
split onto preceding same-engine NoOps (fix_sync_waits).
"""
import sys
sys.path.insert(0, '/opt/trn_rl_repo')
import numpy as np
import ml_dtypes
import concourse.bass as bass
import concourse.mybir as mybir
from concourse import tile
from concourse.bass_utils import run_bass_kernel_spmd

BF16 = ml_dtypes.bfloat16
F32 = mybir.dt.float32
FP16 = mybir.dt.float16
BF = mybir.dt.bfloat16
AF = mybir.ActivationFunctionType

N_CORES = 8
B_LOC = 8192          # batch rows per core
NB = 512              # batch columns per compute chunk (one fp32 PSUM bank)
NCHUNK = B_LOC // NB
NB_LOAD = 1024        # batch columns per x DMA super-tile
K1 = 784
KTILES = [(k, min(128, K1 - k)) for k in range(0, K1, 128)]  # 6x128 + 16
F1, F2, F3, F4 = 256, 128, 32, 10
MAX_WAITS = 1
PASS_DT = ((FP16, np.float16), (FP16, np.float16))  # L1: fp16 hi, fp16 residual


def fix_sync_waits(nc):
    for fn in nc.m.functions:
        for bb in fn.blocks:
            out = []
            changed = False
            for ins in bb.instructions:
                si = ins.sync_info
                waits = list(si.on_wait) if si is not None else []
                if len(waits) > MAX_WAITS:
                    head, keep = waits[:-MAX_WAITS], waits[-MAX_WAITS:]
                    k = 0
                    while head:
                        chunk, head = head[:MAX_WAITS], head[MAX_WAITS:]
                        nop = mybir.InstNoOp(
                            name=f"{ins.name}-wsplit{k}", engine=ins.engine)
                        nop.sync_info = mybir.SyncInfo(on_wait=chunk, on_update=[])
                        out.append(nop)
                        k += 1
                    ins.sync_info = mybir.SyncInfo(
                        on_wait=keep, on_update=list(si.on_update))
                    changed = True
                out.append(ins)
            if changed:
                bb.instructions = out


def round_mant11(a):
    """fp32 -> RNE at 11 explicit mantissa bits (= HW f32r input rounding)."""
    u = np.ascontiguousarray(a).view(np.uint32)
    drop = 12
    lsb = ((u >> drop) & 1).astype(np.uint32)
    r = ((u + np.uint32((1 << (drop - 1)) - 1) + lsb) >> drop) << drop
    return r.view(np.float32)


def build_nc(rep=1):
    nc = bass.Bass()
    x_d = nc.declare_dram_parameter("xqs", [K1, 2, B_LOC], FP16, isOutput=False)
    w1_d = nc.declare_dram_parameter("w1sT", [K1, F1], FP16, isOutput=False)
    w2_d = nc.declare_dram_parameter("w2sT", [F1, F2], BF, isOutput=False)
    w3_d = nc.declare_dram_parameter("w3sT", [F2, F3], BF, isOutput=False)
    w4_d = nc.declare_dram_parameter("w4sT", [F3, F4], BF, isOutput=False)
    out_d = nc.declare_dram_parameter("out", [F4, B_LOC], F32, isOutput=True)

    with tile.TileContext(nc) as tc:
        with tc.tile_pool(name="wpool", bufs=1) as wpool, \
             tc.tile_pool(name="xtpool", bufs=2) as xtpool, \
             tc.tile_pool(name="apool", bufs=2) as apool, \
             tc.tile_pool(name="opool", bufs=2) as opool, \
             tc.tile_pool(name="ps1", bufs=2, space="PSUM") as ps1, \
             tc.tile_pool(name="ps2", bufs=2, space="PSUM") as ps2, \
             tc.tile_pool(name="ps34", bufs=1, space="PSUM") as ps34:
            # head: interleave weight k-tiles with the first x super-group so
            # the first matmul's operands land back-to-back. Full k-tiles load
            # hi|res combined in ONE DMA ([kw, 2*NB_LOAD]); the 16-row k-tail
            # loads hi into partitions 0:16 and res into 16:32 of one [32,*]
            # tile so both passes' tails run as a single 32-contraction matmul.
            NKF = len(KTILES) - 1           # full 128-row k-tiles
            TK0, TKW = KTILES[-1]           # 768, 16
            w1_t = [None] * NKF
            w1tail = None
            xg0 = []

            def load_group(g, r):
                gb0 = (g % (B_LOC // NB_LOAD)) * NB_LOAD
                eng = nc.sync if g % 2 == 0 else nc.scalar
                row = []
                for i, (k0, kw) in enumerate(KTILES[:NKF]):
                    t = xtpool.tile([kw, 2 * NB_LOAD], FP16,
                                    name=f"xG_{r}_{g}_{i}", tag=f"xG{i}")
                    eng.dma_start(t[:], x_d[k0:k0 + kw, :, gb0:gb0 + NB_LOAD])
                    row.append(t)
                tt = xtpool.tile([32, NB_LOAD], FP16,
                                 name=f"xGt_{r}_{g}", tag="xGt")
                eng.dma_start(tt[:TKW, :], x_d[TK0:TK0 + TKW, 0, gb0:gb0 + NB_LOAD])
                eng.dma_start(tt[16:16 + TKW, :],
                              x_d[TK0:TK0 + TKW, 1, gb0:gb0 + NB_LOAD])
                row.append(tt)
                return row

            for i, (k0, kw) in enumerate(KTILES[:NKF]):
                t = wpool.tile([kw, F1], FP16, name=f"w1t_{i}")
                nc.sync.dma_start(t[:], w1_d[k0:k0 + kw, :])
                w1_t[i] = t
                if i < 2:
                    if i == 0:
                        xg0 = load_group(0, 0)
            w1tail = wpool.tile([32, F1], FP16, name="w1tail")
            nc.sync.dma_start(w1tail[:TKW, :], w1_d[TK0:TK0 + TKW, :])
            nc.sync.dma_start(w1tail[16:16 + TKW, :], w1_d[TK0:TK0 + TKW, :])
            w2_t = []
            for i in range(2):
                t = wpool.tile([128, F2], BF, name=f"w2t{i}")
                nc.scalar.dma_start(t[:], w2_d[i * 128:(i + 1) * 128, :])
                w2_t.append(t)
            w3_t = wpool.tile([F2, F3], BF, name="w3t")
            nc.scalar.dma_start(w3_t[:], w3_d[:, :])
            w4_t = wpool.tile([F3, F4], BF, name="w4t")
            nc.scalar.dma_start(w4_t[:], w4_d[:, :])
            zb = wpool.tile([128, 1], F32, name="zb")
            nc.vector.memset(zb[:], 0.0)
            hb = wpool.tile([128, 1], F32, name="hb")
            nc.vector.memset(hb[:], 0.5)

            nsub = NB_LOAD // NB
            xg = None
            for r in range(rep):
                for c in range(NCHUNK):
                    b0 = c * NB
                    g, j = divmod(c, nsub)
                    if j == 0:
                        xg = xg0 if (r == 0 and g == 0) else load_group(g, r)
                    # rhs slices for this 512-chunk: [hi | res] halves per tile
                    rhs0 = [t[:, j * NB:(j + 1) * NB] for t in xg[:NKF]]
                    rhs1 = [t[:, NB_LOAD + j * NB: NB_LOAD + (j + 1) * NB]
                            for t in xg[:NKF]]
                    rhs_tail = xg[NKF][:, j * NB:(j + 1) * NB]
                    a1 = []
                    for f in range(2):
                        p1 = ps1.tile([128, NB], F32, name=f"p1_{r}_{c}_{f}",
                                      tag="p1")
                        fs = slice(f * 128, (f + 1) * 128)
                        for i in range(NKF):
                            nc.tensor.matmul(p1[:], w1_t[i][:, fs], rhs0[i],
                                             start=(i == 0), stop=False)
                        for i in range(NKF):
                            nc.tensor.matmul(p1[:], w1_t[i][:, fs], rhs1[i],
                                             start=False, stop=False)
                        nc.tensor.matmul(p1[:], w1tail[:, fs], rhs_tail,
                                         start=False, stop=True)
                        s1 = apool.tile([128, NB], BF, name=f"a1_{r}_{c}_{f}",
                                        tag=f"a1{f}")
                        nc.scalar.activation(s1[:], p1[:], AF.Sign, bias=zb[:],
                                             scale=1.0)
                        a1.append(s1)
                    p2 = ps2.tile([F2, NB], F32, name=f"p2_{r}_{c}", tag="p2")
                    nc.tensor.matmul(p2[:], w2_t[0][:], a1[0][:], start=True,
                                     stop=False)
                    nc.tensor.matmul(p2[:], w2_t[1][:], a1[1][:], start=False,
                                     stop=True)
                    a2 = apool.tile([F2, NB], BF, name=f"a2_{r}_{c}", tag="a2")
                    nc.scalar.activation(a2[:], p2[:], AF.Sign, bias=hb[:],
                                         scale=1.0)
                    p3 = ps34.tile([F3, NB], F32, name=f"p3_{r}_{c}", tag="p3")
                    nc.tensor.matmul(p3[:], w3_t[:], a2[:], start=True, stop=True)
                    a3 = apool.tile([F3, NB], BF, name=f"a3_{r}_{c}", tag="a3")
                    nc.scalar.activation(a3[:], p3[:], AF.Sign, bias=hb[:F3, :],
                                         scale=1.0)
                    p4 = ps34.tile([F4, NB], F32, name=f"p4_{r}_{c}", tag="p4")
                    nc.tensor.matmul(p4[:], w4_t[:], a3[:], start=True, stop=True)
                    o = opool.tile([F4, NB], F32, name=f"o_{r}_{c}", tag="o")
                    nc.vector.tensor_copy(o[:], p4[:])
                    nc.sync.dma_start(out_d[:, b0:b0 + NB], o[:])
    fix_sync_waits(nc)
    return nc


def _sg(w):
    return np.where(w >= 0, np.float32(1.0), np.float32(-1.0))


_NC_CACHE = {}


def kernel(x, w1, w2, w3, w4):
    if "nc" not in _NC_CACHE:
        _NC_CACHE["nc"] = build_nc()
    nc = _NC_CACHE["nc"]

    x = np.ascontiguousarray(np.asarray(x).reshape(-1, K1), dtype=np.float32)
    w1sT = np.ascontiguousarray(_sg(np.asarray(w1)).T)
    wm = {
        "w1sT": w1sT.astype(np.float16),       # +-1 exact in fp16
        "w2sT": np.ascontiguousarray(_sg(np.asarray(w2)).T).astype(BF16),
        "w3sT": np.ascontiguousarray(_sg(np.asarray(w3)).T).astype(BF16),
        "w4sT": np.ascontiguousarray(_sg(np.asarray(w4)).T).astype(BF16),
    }
    xq = x.astype(np.float16)
    xs = (x - xq.astype(np.float32)).astype(np.float16)
    xqs = np.empty((K1, 2, x.shape[0]), np.float16)   # [784, 2, 65536]
    xqs[:, 0, :] = xq.T
    xqs[:, 1, :] = xs.T

    maps = []
    for c in range(N_CORES):
        m = dict(wm)
        m["xqs"] = xqs[:, :, c * B_LOC:(c + 1) * B_LOC]
        maps.append(m)

    res = None
    last_exc = None
    for attempt in range(3):
        try:
            res = run_bass_kernel_spmd(nc, maps, list(range(N_CORES)))
            break
        except Exception as e:  # transient NRT/device errors: retry
            last_exc = e
            import time
            time.sleep(5 * (attempt + 1))
    if res is None:
        raise last_exc
    outs = [r["out"] for r in res.results]                 # [10, 8192] each
    return np.ascontiguousarray(
        np.concatenate([o.T for o in outs], axis=0)).astype(np.float32)


# revision 6
# speedup vs baseline: 1.1929x; 1.1032x over previous
"""Trainium2 Bass kernel: binarized-MLP forward (784-256-128-32-10, ste_sign).

Strategy
--------
Pure data parallel over 8 NeuronCores: batch 65536 -> 8 shards of 8192 rows;
the tiny sign-binarized weights are replicated (binarized + transposed on the
host). Each core runs the full 4-layer network on its shard; outputs are
gathered on the host. No collectives needed (forward only).

On-chip the network runs feature-major: activations live as [features, batch]
tiles and every matmul streams batch as the moving dimension, so layer N's
output feeds layer N+1 with no transposes between layers. x is pre-transposed
to [784, B] on the host so the contraction dim lands on the partition axis
straight out of DMA.

Layer 1 (x is real-valued fp32; everything downstream only sees sign(h1)) uses
a Dekker-style two-pass fp16 split: the host ships xh = fp16(x) plus the
fp16-rounded residual (x - xh), and both passes (weights +-1, exact in fp16)
accumulate into one PSUM group at 1 cycle/row each (vs 4 for native fp32).
This recovers ~21 mantissa bits — beyond the fp32 reference's own rounding
noise (measured on HW: max err 2.1e-5 on 784-length dots, 0 sign flips in
65536) — at half the PE cost of fp32 and 4 bytes/element of DMA (fp32's
bandwidth) instead of 6 for an f32r+bf16 split.

Layers 2-4 have +-1 inputs and +-1 weights, so bf16 matmuls are exact
(integer partial sums <= 256); the sign activations run on the scalar engine.
ACT Sign(0) = 0 on this HW, so integer-valued pre-activations (layers 2,3) use
Sign(h + 0.5), which reproduces the reference's sign(0)=+1 exactly. The final
logits are integers in [-32, 32], computed exactly.

x is loaded in [k-tile, 1024]-column super-tiles (fewer, larger DMAs — the
DMA queue is the second-busiest resource), split across both HWDGE engines
(xq on SP, residual on ACT), with the first super-group prefetched ahead of
the weight loads so the PE starts early.

This walrus build rejects instructions carrying more than one semaphore wait
("Too many sync wait commands"), so after Tile scheduling, excess waits are
split onto preceding same-engine NoOps (fix_sync_waits).
"""
import sys
sys.path.insert(0, '/opt/trn_rl_repo')
import numpy as np
import ml_dtypes
import concourse.bass as bass
import concourse.mybir as mybir
from concourse import tile
from concourse.bass_utils import run_bass_kernel_spmd

BF16 = ml_dtypes.bfloat16
F32 = mybir.dt.float32
FP16 = mybir.dt.float16
BF = mybir.dt.bfloat16
AF = mybir.ActivationFunctionType

N_CORES = 8
B_LOC = 8192          # batch rows per core
NB = 512              # batch columns per compute chunk (one fp32 PSUM bank)
NCHUNK = B_LOC // NB
NB_LOAD = 1024        # batch columns per x DMA super-tile
K1 = 784
KTILES = [(k, min(128, K1 - k)) for k in range(0, K1, 128)]  # 6x128 + 16
F1, F2, F3, F4 = 256, 128, 32, 10
MAX_WAITS = 1
PASS_DT = ((FP16, np.float16), (FP16, np.float16))  # L1: fp16 hi, fp16 residual


def fix_sync_waits(nc):
    for fn in nc.m.functions:
        for bb in fn.blocks:
            out = []
            changed = False
            for ins in bb.instructions:
                si = ins.sync_info
                waits = list(si.on_wait) if si is not None else []
                if len(waits) > MAX_WAITS:
                    head, keep = waits[:-MAX_WAITS], waits[-MAX_WAITS:]
                    k = 0
                    while head:
                        chunk, head = head[:MAX_WAITS], head[MAX_WAITS:]
                        nop = mybir.InstNoOp(
                            name=f"{ins.name}-wsplit{k}", engine=ins.engine)
                        nop.sync_info = mybir.SyncInfo(on_wait=chunk, on_update=[])
                        out.append(nop)
                        k += 1
                    ins.sync_info = mybir.SyncInfo(
                        on_wait=keep, on_update=list(si.on_update))
                    changed = True
                out.append(ins)
            if changed:
                bb.instructions = out


def round_mant11(a):
    """fp32 -> RNE at 11 explicit mantissa bits (= HW f32r input rounding)."""
    u = np.ascontiguousarray(a).view(np.uint32)
    drop = 12
    lsb = ((u >> drop) & 1).astype(np.uint32)
    r = ((u + np.uint32((1 << (drop - 1)) - 1) + lsb) >> drop) << drop
    return r.view(np.float32)


def build_nc(rep=1):
    nc = bass.Bass()
    x_d = nc.declare_dram_parameter("xqs", [K1, 2, B_LOC], FP16, isOutput=False)
    w1_d = nc.declare_dram_parameter("w1sT", [K1, F1], FP16, isOutput=False)
    w2_d = nc.declare_dram_parameter("w2sT", [F1, F2], BF, isOutput=False)
    w3_d = nc.declare_dram_parameter("w3sT", [F2, F3], BF, isOutput=False)
    w4_d = nc.declare_dram_parameter("w4sT", [F3, F4], BF, isOutput=False)
    out_d = nc.declare_dram_parameter("out", [F4, B_LOC], F32, isOutput=True)

    with tile.TileContext(nc) as tc:
        with tc.tile_pool(name="wpool", bufs=1) as wpool, \
             tc.tile_pool(name="xtpool", bufs=2) as xtpool, \
             tc.tile_pool(name="apool", bufs=2) as apool, \
             tc.tile_pool(name="opool", bufs=2) as opool, \
             tc.tile_pool(name="ps1", bufs=2, space="PSUM") as ps1, \
             tc.tile_pool(name="ps2", bufs=2, space="PSUM") as ps2, \
             tc.tile_pool(name="ps34", bufs=1, space="PSUM") as ps34:
            # head: interleave weight k-tiles with the first x super-group so
            # the first matmul's operands land back-to-back. Full k-tiles load
            # hi|res combined in ONE DMA ([kw, 2*NB_LOAD]); the 16-row k-tail
            # loads hi into partitions 0:16 and res into 16:32 of one [32,*]
            # tile so both passes' tails run as a single 32-contraction matmul.
            NKF = len(KTILES) - 1           # full 128-row k-tiles
            TK0, TKW = KTILES[-1]           # 768, 16
            w1_t = [None] * NKF
            w1tail = None
            xg0 = []

            def load_group(g, r):
                gb0 = (g % (B_LOC // NB_LOAD)) * NB_LOAD
                row = []
                for i, (k0, kw) in enumerate(KTILES[:NKF]):
                    t = xtpool.tile([kw, 2 * NB_LOAD], FP16,
                                    name=f"xG_{r}_{g}_{i}", tag=f"xG{i}")
                    eng = nc.sync if (g + i) % 2 == 0 else nc.scalar
                    eng.dma_start(t[:], x_d[k0:k0 + kw, :, gb0:gb0 + NB_LOAD])
                    row.append(t)
                tt = xtpool.tile([32, NB_LOAD], FP16,
                                 name=f"xGt_{r}_{g}", tag="xGt")
                eng = nc.sync if g % 2 == 0 else nc.scalar
                eng.dma_start(tt[:TKW, :], x_d[TK0:TK0 + TKW, 0, gb0:gb0 + NB_LOAD])
                eng.dma_start(tt[16:16 + TKW, :],
                              x_d[TK0:TK0 + TKW, 1, gb0:gb0 + NB_LOAD])
                row.append(tt)
                return row

            # head: pair each weight k-tile with its group-0 x tile, spread
            # across both HWDGE queues so the accumulation chain's operands
            # land in matmul order.
            gb0 = 0
            for i, (k0, kw) in enumerate(KTILES[:NKF]):
                eng = nc.sync if i % 2 == 0 else nc.scalar
                t = wpool.tile([kw, F1], FP16, name=f"w1t_{i}")
                eng.dma_start(t[:], w1_d[k0:k0 + kw, :])
                w1_t[i] = t
                tx = xtpool.tile([kw, 2 * NB_LOAD], FP16,
                                 name=f"xG_0_0_{i}", tag=f"xG{i}")
                eng.dma_start(tx[:], x_d[k0:k0 + kw, :, 0:NB_LOAD])
                xg0.append(tx)
            w1tail = wpool.tile([32, F1], FP16, name="w1tail")
            nc.sync.dma_start(w1tail[:TKW, :], w1_d[TK0:TK0 + TKW, :])
            nc.scalar.dma_start(w1tail[16:16 + TKW, :], w1_d[TK0:TK0 + TKW, :])
            tt0 = xtpool.tile([32, NB_LOAD], FP16, name="xGt_0_0", tag="xGt")
            nc.sync.dma_start(tt0[:TKW, :], x_d[TK0:TK0 + TKW, 0, 0:NB_LOAD])
            nc.scalar.dma_start(tt0[16:16 + TKW, :],
                                x_d[TK0:TK0 + TKW, 1, 0:NB_LOAD])
            xg0.append(tt0)
            w2_t = []
            for i in range(2):
                t = wpool.tile([128, F2], BF, name=f"w2t{i}")
                nc.scalar.dma_start(t[:], w2_d[i * 128:(i + 1) * 128, :])
                w2_t.append(t)
            w3_t = wpool.tile([F2, F3], BF, name="w3t")
            nc.scalar.dma_start(w3_t[:], w3_d[:, :])
            w4_t = wpool.tile([F3, F4], BF, name="w4t")
            nc.scalar.dma_start(w4_t[:], w4_d[:, :])
            zb = wpool.tile([128, 1], F32, name="zb")
            nc.vector.memset(zb[:], 0.0)
            hb = wpool.tile([128, 1], F32, name="hb")
            nc.vector.memset(hb[:], 0.5)

            nsub = NB_LOAD // NB
            xg = None
            for r in range(rep):
                for c in range(NCHUNK):
                    b0 = c * NB
                    g, j = divmod(c, nsub)
                    if j == 0:
                        xg = xg0 if (r == 0 and g == 0) else load_group(g, r)
                    # rhs slices for this 512-chunk: [hi | res] halves per tile
                    rhs0 = [t[:, j * NB:(j + 1) * NB] for t in xg[:NKF]]
                    rhs1 = [t[:, NB_LOAD + j * NB: NB_LOAD + (j + 1) * NB]
                            for t in xg[:NKF]]
                    rhs_tail = xg[NKF][:, j * NB:(j + 1) * NB]
                    a1 = []
                    for f in range(2):
                        p1 = ps1.tile([128, NB], F32, name=f"p1_{r}_{c}_{f}",
                                      tag="p1")
                        fs = slice(f * 128, (f + 1) * 128)
                        for i in range(NKF):
                            nc.tensor.matmul(p1[:], w1_t[i][:, fs], rhs0[i],
                                             start=(i == 0), stop=False)
                        for i in range(NKF):
                            nc.tensor.matmul(p1[:], w1_t[i][:, fs], rhs1[i],
                                             start=False, stop=False)
                        nc.tensor.matmul(p1[:], w1tail[:, fs], rhs_tail,
                                         start=False, stop=True)
                        s1 = apool.tile([128, NB], BF, name=f"a1_{r}_{c}_{f}",
                                        tag=f"a1{f}")
                        nc.scalar.activation(s1[:], p1[:], AF.Sign, bias=zb[:],
                                             scale=1.0)
                        a1.append(s1)
                    p2 = ps2.tile([F2, NB], F32, name=f"p2_{r}_{c}", tag="p2")
                    nc.tensor.matmul(p2[:], w2_t[0][:], a1[0][:], start=True,
                                     stop=False)
                    nc.tensor.matmul(p2[:], w2_t[1][:], a1[1][:], start=False,
                                     stop=True)
                    a2 = apool.tile([F2, NB], BF, name=f"a2_{r}_{c}", tag="a2")
                    nc.scalar.activation(a2[:], p2[:], AF.Sign, bias=hb[:],
                                         scale=1.0)
                    p3 = ps34.tile([F3, NB], F32, name=f"p3_{r}_{c}", tag="p3")
                    nc.tensor.matmul(p3[:], w3_t[:], a2[:], start=True, stop=True)
                    a3 = apool.tile([F3, NB], BF, name=f"a3_{r}_{c}", tag="a3")
                    nc.scalar.activation(a3[:], p3[:], AF.Sign, bias=hb[:F3, :],
                                         scale=1.0)
                    p4 = ps34.tile([F4, NB], F32, name=f"p4_{r}_{c}", tag="p4")
                    nc.tensor.matmul(p4[:], w4_t[:], a3[:], start=True, stop=True)
                    o = opool.tile([F4, NB], F32, name=f"o_{r}_{c}", tag="o")
                    nc.vector.tensor_copy(o[:], p4[:])
                    nc.sync.dma_start(out_d[:, b0:b0 + NB], o[:])
    fix_sync_waits(nc)
    return nc


def _sg(w):
    return np.where(w >= 0, np.float32(1.0), np.float32(-1.0))


_NC_CACHE = {}


def kernel(x, w1, w2, w3, w4):
    if "nc" not in _NC_CACHE:
        _NC_CACHE["nc"] = build_nc()
    nc = _NC_CACHE["nc"]

    x = np.ascontiguousarray(np.asarray(x).reshape(-1, K1), dtype=np.float32)
    w1sT = np.ascontiguousarray(_sg(np.asarray(w1)).T)
    wm = {
        "w1sT": w1sT.astype(np.float16),       # +-1 exact in fp16
        "w2sT": np.ascontiguousarray(_sg(np.asarray(w2)).T).astype(BF16),
        "w3sT": np.ascontiguousarray(_sg(np.asarray(w3)).T).astype(BF16),
        "w4sT": np.ascontiguousarray(_sg(np.asarray(w4)).T).astype(BF16),
    }
    xq = x.astype(np.float16)
    xs = (x - xq.astype(np.float32)).astype(np.float16)
    xqs = np.empty((K1, 2, x.shape[0]), np.float16)   # [784, 2, 65536]
    xqs[:, 0, :] = xq.T
    xqs[:, 1, :] = xs.T

    maps = []
    for c in range(N_CORES):
        m = dict(wm)
        m["xqs"] = xqs[:, :, c * B_LOC:(c + 1) * B_LOC]
        maps.append(m)

    res = None
    last_exc = None
    for attempt in range(3):
        try:
            res = run_bass_kernel_spmd(nc, maps, list(range(N_CORES)))
            break
        except Exception as e:  # transient NRT/device errors: retry
            last_exc = e
            import time
            time.sleep(5 * (attempt + 1))
    if res is None:
        raise last_exc
    outs = [r["out"] for r in res.results]                 # [10, 8192] each
    return np.ascontiguousarray(
        np.concatenate([o.T for o in outs], axis=0)).astype(np.float32)
